# revision 8
# baseline (speedup 1.0000x reference)
"""BoxBottleneck kernel for 8 Trainium2 NeuronCores.

Pipeline: 1x1 conv (Cin=256 -> 16) + BN + ReLU -> learnable box filter
(integral image + bilinear corners) -> BN + ReLU -> 1x1 conv (64 -> 256)
+ BN -> ReLU(out + x).

Key algebraic transform: the box filter for channel c / box b is a
separable linear map on the 56x56 plane:
    out_plane = P[c,b] @ plane @ Q[c,b]
where P and Q fold the cumsum (triangular) matrices and the bilinear
corner interpolation.  Both collapse to clamp form:
    P[c,b][i,j] = clamp(y2_i - j, 0, 1) - clamp(y1_i - j, 0, 1)
(and transposed likewise for Q), so instead of shipping the dense
56x56 matrices per (c,b), the kernel ships only the clipped endpoint
vectors and materializes P^T (BN2-scale folded) and Q on device with a
handful of DVE ops.  BN scales fold into the adjacent matmul weights;
BN biases ride along as an extra contraction row (ones-row trick) or as
per-partition activation bias.

Sharding: pure data parallel, 4 samples per core.

The axon tunnel to the cores moves ~50 MB/s half-duplex, so the call
time is dominated by wire bytes, not device compute.  Wire plan: x/y
travel as fp16 (x: 51.4 MB up, y: 51.4 MB down), consts are ~130 KB per
core, the output-donation buffers are created on device instead of
being shipped as zeros, and the jitted shard_map executable is built
once and cached across calls (the library path rebuilds and reloads it
per call).
"""

import sys

sys.path.insert(0, "/opt/trn_rl_repo")

import numpy as np

N, CIN, H, W = 32, 256, 56, 56
CMID, B = 16, 4
CBOX, COUT = 64, 256
HW = H * W
NCORES = 8
NPC = N // NCORES
EPS = 1e-5

_CACHE = {}


def _blob_layout(spec):
    out, off = {}, 0
    for name, ln in spec:
        out[name] = (off, ln)
        off += ln
    return out, off


BLOB32, BLOB32_LEN = _blob_layout([
    ("b1p", CMID), ("iotap", 128),
    ("x2v", CMID * 256), ("x1v", CMID * 256),
    ("y2v", CBOX * 56), ("y1v", CBOX * 56),
    ("srow", CBOX * 56), ("b2p", CBOX * 56),
    ("ones", CMID * 224),
])
BLOB16, BLOB16_LEN = _blob_layout([
    ("w1t", 128 * 2 * CMID), ("w3t", (CBOX + 1) * COUT), ("onesr", HW),
])


def _build_nc():
    import concourse.mybir as mybir
    import concourse.tile as tile
    from concourse import bacc

    f16 = mybir.dt.float16
    f32 = mybir.dt.float32
    RELU = mybir.ActivationFunctionType.Relu

    nc = bacc.Bacc("TRN2", target_bir_lowering=False, debug=False, num_devices=NCORES)

    xin = nc.declare_dram_parameter("xin", [NPC, 2, 128, HW], f16, isOutput=False)
    cb32 = nc.declare_dram_parameter("cb32", [1, BLOB32_LEN], f32, isOutput=False)
    cb16 = nc.declare_dram_parameter("cb16", [1, BLOB16_LEN], f16, isOutput=False)
    y = nc.declare_dram_parameter("y", [NPC, 2, 128, HW], f16, isOutput=True)

    def s32(name):
        o, ln = BLOB32[name]
        return cb32[0:1, o : o + ln]

    def s16(name):
        o, ln = BLOB16[name]
        return cb16[0:1, o : o + ln]

    NT = 7  # free-dim tiles of 448 over 3136 pixels

    with tile.TileContext(nc) as tc:
        with (
            tc.tile_pool(name="const", bufs=1) as cpool,
            tc.tile_pool(name="seg", bufs=3) as segp,
            tc.tile_pool(name="xp", bufs=4) as xpool,
            tc.tile_pool(name="midp", bufs=1) as midpool,
            tc.tile_pool(name="mtp", bufs=2) as mtpool,
            tc.tile_pool(name="tcp", bufs=2) as tcpool,
            tc.tile_pool(name="usp", bufs=2) as upool,
            tc.tile_pool(name="zp", bufs=1) as zpool,
            tc.tile_pool(name="outp", bufs=4) as outpool,
            tc.tile_pool(name="o16p", bufs=2) as o16pool,
            tc.tile_pool(name="drm", bufs=4, space="DRAM") as drmpool,
            tc.tile_pool(name="dru", bufs=4, space="DRAM") as drupool,
            tc.tile_pool(name="ps1", bufs=2, space="PSUM") as ps1,
            tc.tile_pool(name="ps2", bufs=2, space="PSUM") as ps2,
            tc.tile_pool(name="ps3", bufs=2, space="PSUM") as ps3,
            tc.tile_pool(name="ps4", bufs=2, space="PSUM") as ps4,
        ):
            ALU = mybir.AluOpType
            w1s = cpool.tile([128, 2 * CMID], f16)
            nc.sync.dma_start(
                w1s[:], s16("w1t").rearrange("o (p c) -> (o p) c", p=128)
            )
            b1s = cpool.tile([CMID, 1], f32)
            nc.sync.dma_start(
                b1s[:], s32("b1p").rearrange("o (p c) -> (o p) c", p=CMID)
            )
            w3s = cpool.tile([CBOX + 1, COUT], f16)
            nc.sync.dma_start(
                w3s[:], s16("w3t").rearrange("o (p c) -> (o p) c", p=CBOX + 1)
            )
            iot = cpool.tile([128, 1], f32)
            nc.sync.dma_start(
                iot[:], s32("iotap").rearrange("o (p c) -> (o p) c", p=128)
            )

            def replicate(dst, src_ap, width):
                # fill dst[0:56, 0:width] with copies of the DRAM row via
                # log2 doubling in SBUF
                nc.sync.dma_start(dst[0:1, 0:width], src_ap)
                k = 1
                while k < 56:
                    step = min(k, 56 - k)
                    nc.sync.dma_start(
                        dst[k : k + step, 0:width], dst[0:step, 0:width]
                    )
                    k += step

            # ---- on-device box matrices: Q then P^T (BN2 scale folded) ----
            # Q[x, (c b j)] = clamp(x2[cbj] - x, 0, 1) - clamp(x1[cbj] - x, 0, 1)
            qs = cpool.tile([56, CMID * 256], f16)
            s2t = segp.tile([56, CMID * 256], f32, tag="seg")
            s1t = segp.tile([56, CMID * 256], f32, tag="seg")
            replicate(s2t, s32("x2v"), CMID * 256)
            replicate(s1t, s32("x1v"), CMID * 256)
            nc.vector.tensor_scalar(
                s2t[:], s2t[:], iot[0:56], 0.0, ALU.subtract, ALU.max
            )
            nc.vector.tensor_scalar(
                s1t[:], s1t[:], iot[0:56], 0.0, ALU.subtract, ALU.max
            )
            nc.vector.tensor_scalar(s1t[:], s1t[:], 1.0, None, ALU.min, ALU.bypass)
            nc.vector.scalar_tensor_tensor(
                qs[:], s2t[:], 1.0, s1t[:], ALU.min, ALU.subtract
            )
            # P^T[y, (cb i)] = (clamp(y2[cbi] - y) - clamp(y1[cbi] - y)) * s2/area
            # row 56 carries the BN2 bias (ones-row trick in stage 2)
            psc = cpool.tile([57, CBOX * 56], f32)
            u2t = segp.tile([56, CBOX * 56], f32, tag="seg")
            u1t = segp.tile([56, CBOX * 56], f32, tag="seg")
            srt = segp.tile([56, CBOX * 56], f32, tag="seg")
            replicate(u2t, s32("y2v"), CBOX * 56)
            replicate(u1t, s32("y1v"), CBOX * 56)
            replicate(srt, s32("srow"), CBOX * 56)
            nc.vector.tensor_scalar(
                u2t[:], u2t[:], iot[0:56], 0.0, ALU.subtract, ALU.max
            )
            nc.vector.tensor_scalar(
                u1t[:], u1t[:], iot[0:56], 0.0, ALU.subtract, ALU.max
            )
            nc.vector.tensor_scalar(u1t[:], u1t[:], 1.0, None, ALU.min, ALU.bypass)
            nc.vector.scalar_tensor_tensor(
                u2t[:], u2t[:], 1.0, u1t[:], ALU.min, ALU.subtract
            )
            nc.vector.tensor_tensor(psc[0:56, :], u2t[:], srt[:], ALU.mult)
            nc.sync.dma_start(psc[56:57, :], s32("b2p"))

            for n in range(NPC):
                # ---- load x (two k-chunk fp16 tiles; conv1 consumes fp16) ----
                x_ks = []
                for k in range(2):
                    xk = xpool.tile([128, HW], f16, tag="xk")
                    x_ks.append(xk)
                    nc.sync.dma_start(xk[:], xin[n, k])
                # ---- conv1 (fp16) + bn1-relu, mid stored x-major ----
                mid_t = midpool.tile([CMID, HW], f16)
                mid_xmaj = mid_t[:].rearrange("c (x y) -> c y x", y=56)
                for t in range(NT):
                    pst = ps1.tile([128, 448], f32)
                    for k in range(2):
                        nc.tensor.matmul(
                            pst[0:CMID, :],
                            w1s[:, k * CMID : (k + 1) * CMID],
                            x_ks[k][:, t * 448 : (t + 1) * 448],
                            start=(k == 0),
                            stop=(k == 1),
                        )
                    bn1_dst = mid_xmaj[:, t * 8 : (t + 1) * 8, :]
                    bn1_src = pst[0:CMID, :].rearrange("c (y x) -> c y x", x=56)
                    if t < 4:
                        nc.scalar.activation(bn1_dst, bn1_src, RELU, bias=b1s[:])
                    else:
                        nc.vector.tensor_scalar(
                            bn1_dst, bn1_src, b1s[:], 0.0, ALU.add, ALU.max
                        )
                # ---- layout A via DRAM bounce: dump then scatter-read ----
                scm = drmpool.tile([CMID, HW], f16)
                nc.sync.dma_start(scm[:], mid_t[:])
                midT_t = mtpool.tile([56, CMID * 56], f16)
                nc.sync.dma_start(
                    midT_t[0:56, :].rearrange("x (c y) -> x c y", y=56),
                    scm[:].rearrange("c (x y) -> x c y", y=56),
                )

                # ---- stage 1: Tcol[y, (b j)] = sum_x mid[y,x] Q[x, (b j)] ----
                tcol = tcpool.tile([57, CMID * 224], f32)
                nc.sync.dma_start(tcol[56:57, :], s32("ones"))
                for g in range(8):  # adjacent-c pairs
                    pst = ps2.tile([128, 512], f32)
                    for dc in range(2):
                        c = 2 * g + dc
                        nc.tensor.matmul(
                            pst[0:56, dc * 256 : (dc + 1) * 256],
                            midT_t[0:56, c * 56 : (c + 1) * 56],
                            qs[0:56, c * 256 : (c + 1) * 256],
                            start=True,
                            stop=True,
                        )
                    src = pst[0:56, :].rearrange("p (dc e) -> p dc e", dc=2)[
                        :, :, 0:224
                    ]
                    dst = tcol[0:56, 2 * g * 224 :][:, 0:448]
                    d = dst.rearrange("p (dc e) -> p dc e", dc=2)
                    if g % 2 == 0:
                        nc.scalar.copy(d, src)
                    else:
                        nc.vector.tensor_copy(d, src)

                # ---- stage 2: U[i, j] = sum_y P'[i,y] Tcol[y, (b j)] + bias2 ----
                usb = upool.tile([56, CBOX * 56], f16)
                for kk in range(4):  # two c-pairs per PSUM bank
                    pst = ps3.tile([128, 448], f32)
                    for dc in range(2):
                        cp = 2 * kk + dc
                        for b in range(B):
                            col = dc * 224 + b * 56
                            nc.tensor.matmul(
                                pst[0:56, col : col + 56],
                                psc[0:57, (cp * B + b) * 56 : (cp * B + b + 1) * 56],
                                tcol[0:57, cp * 224 + b * 56 :][:, 0:56],
                                start=True,
                                stop=True,
                            )
                            nc.tensor.matmul(
                                pst[64:120, col : col + 56],
                                psc[
                                    0:57,
                                    ((cp + 8) * B + b) * 56 : ((cp + 8) * B + b + 1)
                                    * 56,
                                ],
                                tcol[0:57, (cp + 8) * 224 + b * 56 :][:, 0:56],
                                start=True,
                                stop=True,
                                tile_position=(0, 64),
                            )
                    # bn2-relu (bias already in matmul via ones row)
                    nc.scalar.activation(
                        usb[0:56, kk * 448 : (kk + 1) * 448], pst[0:56, :], RELU
                    )
                    nc.vector.tensor_scalar(
                        usb[0:56, 1792 + kk * 448 : 1792 + (kk + 1) * 448],
                        pst[64:120, :],
                        0.0,
                        None,
                        ALU.max,
                        ALU.bypass,
                    )

                # ---- layout B + conv3 + bn3 + residual relu ----
                scu = drupool.tile([56, CBOX * 56], f16)
                nc.sync.dma_start(scu[:], usb[0:56, :])
                z_t = zpool.tile([CBOX + 1, HW], f16)
                nc.sync.dma_start(z_t[CBOX : CBOX + 1, :], s16("onesr"))
                nc.sync.dma_start(
                    z_t[0:CBOX, :].rearrange("cb (i j) -> cb i j", j=56),
                    scu[:].rearrange("i (cb j) -> cb i j", j=56),
                )
                for h in range(2):
                    for lo, hi in ((0, 2), (2, 4), (4, 6), (6, 7)):
                        out_t = outpool.tile([128, 896], f32)
                        for t in range(lo, hi):
                            pst = ps4.tile([128, 448], f32)
                            nc.tensor.matmul(
                                pst[:],
                                w3s[:, h * 128 : (h + 1) * 128],
                                z_t[:, t * 448 : (t + 1) * 448],
                                start=True,
                                stop=True,
                            )
                            nc.vector.scalar_tensor_tensor(
                                out_t[:, (t - lo) * 448 : (t - lo + 1) * 448],
                                pst[:],
                                1.0,
                                x_ks[h][:, t * 448 : (t + 1) * 448],
                                ALU.mult,
                                ALU.add,
                            )
                        w = (hi - lo) * 448
                        o16 = o16pool.tile([128, 896], f16, tag="o16")
                        if (h * 4 + lo // 2) % 2 == 0:
                            nc.gpsimd.tensor_scalar(
                                o16[:, 0:w], out_t[:, 0:w], 0.0, None, ALU.max,
                                ALU.bypass,
                            )
                        else:
                            nc.scalar.activation(
                                o16[:, 0:w], out_t[:, 0:w], RELU
                            )
                        nc.sync.dma_start(
                            y[n, h][:, lo * 448 : hi * 448], o16[:, 0:w]
                        )

    nc.compile()
    return nc


def _build_runner(nc):
    """Build the jitted shard_map executable ONCE and reuse across calls.

    Mirrors concourse.bass2jax.run_bass_via_pjrt, but (a) caches the jit
    so repeat calls skip retrace/reload, and (b) materializes the donated
    output buffers on device instead of shipping host zeros over the
    axon tunnel.
    """
    import jax
    import jax.numpy as jnp
    from jax.experimental.shard_map import shard_map
    from jax.sharding import Mesh, NamedSharding, PartitionSpec

    import concourse.mybir as mybir
    from concourse import bass2jax

    bass2jax.install_neuronx_cc_hook()
    assert nc.dbg_addr is None or not nc.dbg_callbacks

    partition_name = nc.partition_id_tensor.name if nc.partition_id_tensor else None

    in_names = []
    out_names = []
    out_avals = []
    for alloc in nc.m.functions[0].allocations:
        if not isinstance(alloc, mybir.MemoryLocationSet):
            continue
        name = alloc.memorylocations[0].name
        if alloc.kind == "ExternalInput":
            if name != partition_name:
                in_names.append(name)
        elif alloc.kind == "ExternalOutput":
            shape = tuple(alloc.tensor_shape)
            dtype = mybir.dt.np(alloc.dtype)
            out_names.append(name)
            out_avals.append(jax.core.ShapedArray(shape, dtype))
    n_params = len(in_names)
    param_names = list(in_names)
    dbg_name = None
    if nc.dbg_addr is not None:
        dbg_name = nc.dbg_addr.name
    in_names = in_names + out_names
    if partition_name is not None:
        in_names = in_names + [partition_name]

    donate = tuple(range(n_params, n_params + len(out_names)))

    def _body(*args):
        operands = list(args)
        if partition_name is not None:
            operands.append(bass2jax.partition_id_tensor())
        outs = bass2jax._bass_exec_p.bind(
            *operands,
            out_avals=tuple(out_avals),
            in_names=tuple(in_names),
            out_names=tuple(out_names),
            lowering_input_output_aliases=(),
            sim_require_finite=True,
            sim_require_nnan=True,
            nc=nc,
        )
        return tuple(outs)

    devices = jax.devices()[:NCORES]
    mesh = Mesh(np.asarray(devices), ("core",))
    n_io = n_params + len(out_names)
    sharded = jax.jit(
        shard_map(
            _body,
            mesh=mesh,
            in_specs=(PartitionSpec("core"),) * n_io,
            out_specs=(PartitionSpec("core"),) * len(out_names),
            check_rep=False,
        ),
        donate_argnums=donate,
        keep_unused=True,
    )
    out_sh = NamedSharding(mesh, PartitionSpec("core"))
    zeros_fns = []
    for av in out_avals:
        gshape = (NCORES * av.shape[0], *av.shape[1:])
        zeros_fns.append(
            jax.jit(
                lambda shape=gshape, dt=av.dtype: jnp.zeros(shape, dt),
                out_shardings=out_sh,
            )
        )
    return {
        "sharded": sharded,
        "zeros_fns": zeros_fns,
        "param_names": param_names,
        "out_names": out_names,
        "out_avals": out_avals,
        "dbg_name": dbg_name,
    }


def _prepare_consts(inputs):
    f8 = np.float64
    g1, b1, m1, v1 = (inputs[k].astype(f8) for k in ("g1", "b1", "m1", "v1"))
    g2, b2, m2, v2 = (inputs[k].astype(f8) for k in ("g2", "b2", "m2", "v2"))
    g3, b3, m3, v3 = (inputs[k].astype(f8) for k in ("g3", "b3", "m3", "v3"))
    s1 = g1 / np.sqrt(v1 + EPS)
    s2 = g2 / np.sqrt(v2 + EPS)
    s3 = g3 / np.sqrt(v3 + EPS)
    b1v = b1 - m1 * s1
    b2v = b2 - m2 * s2
    b3v = b3 - m3 * s3
    w1p = inputs["w1"].astype(f8) * s1[:, None]
    w3p = inputs["w3"].astype(f8) * s3[:, None]

    y_min, y_max, x_min, x_max = (
        inputs[k].astype(f8) for k in ("y_min", "y_max", "x_min", "x_max")
    )
    area = (y_max - y_min + 1.0) * (x_max - x_min + 1.0)  # (C, B)
    idx = np.arange(W, dtype=f8)

    # clamp-form endpoint vectors (see module docstring)
    x2m = np.clip(idx[None, None, :] + x_max[:, :, None] + 1.0, 0.0, W)  # (C,B,56)
    x1m = np.clip(idx[None, None, :] + x_min[:, :, None], 0.0, W)
    pad = np.zeros((CMID, 32), f8)
    x2v = np.concatenate([x2m.reshape(CMID, B * 56), pad], axis=1).reshape(1, -1)
    x1v = np.concatenate([x1m.reshape(CMID, B * 56), pad], axis=1).reshape(1, -1)

    y2m = np.clip(idx[None, None, :] + y_max[:, :, None] + 1.0, 0.0, H)
    y1m = np.clip(idx[None, None, :] + y_min[:, :, None], 0.0, H)
    y2v = y2m.reshape(1, CBOX * 56)
    y1v = y1m.reshape(1, CBOX * 56)
    sm = (s2.reshape(CMID, B) / area)[:, :, None] * np.ones((1, 1, 56), f8)
    srow = sm.reshape(1, CBOX * 56)
    b2m = b2v.reshape(CMID, B)[:, :, None] * np.ones((1, 1, 56), f8)
    b2p = b2m.reshape(1, CBOX * 56)

    w1t = np.zeros((128, 2 * CMID), np.float16)
    for k in range(2):
        w1t[:, k * CMID : (k + 1) * CMID] = w1p[:, k * 128 : (k + 1) * 128].T

    w3t = np.zeros((CBOX + 1, COUT), np.float16)
    w3t[0:CBOX, :] = w3p.T
    w3t[CBOX, :] = b3v
    f4 = np.float32
    parts32 = {
        "b1p": b1v.astype(f4).ravel(),
        "iotap": np.arange(128, dtype=f4),
        "x2v": x2v.astype(f4).ravel(), "x1v": x1v.astype(f4).ravel(),
        "y2v": y2v.astype(f4).ravel(), "y1v": y1v.astype(f4).ravel(),
        "srow": srow.astype(f4).ravel(), "b2p": b2p.astype(f4).ravel(),
        "ones": np.ones(CMID * 224, f4),
    }
    blob32 = np.zeros((1, BLOB32_LEN), f4)
    for name, (off, ln) in BLOB32.items():
        blob32[0, off : off + ln] = parts32[name]
    parts16 = {
        "w1t": w1t.ravel(),
        "w3t": w3t.ravel(),
        "onesr": np.ones(HW, np.float16),
    }
    blob16 = np.zeros((1, BLOB16_LEN), np.float16)
    for name, (off, ln) in BLOB16.items():
        blob16[0, off : off + ln] = parts16[name]
    return {"cb32": blob32, "cb16": blob16}


def _host_prep(inputs):
    """All host-side marshalling: const folding + fp16 staging + per-core
    replication of the small params.  Returns the GLOBAL (concat-on-axis-0)
    arrays the sharded executable consumes."""
    consts = _prepare_consts(inputs)
    x = np.asarray(inputs["x"])
    xg = np.ascontiguousarray(x.reshape(N, 2, 128, HW)).astype(np.float16)
    g = {"xin": xg}
    for k, v in consts.items():
        g[k] = np.concatenate([v] * NCORES, axis=0)
    return g


def _host_post(y16):
    return y16.reshape(N, COUT, H, W).astype(np.float32)


def kernel(**inputs):
    if "runner" not in _CACHE:
        _CACHE["nc"] = _build_nc()
        _CACHE["runner"] = _build_runner(_CACHE["nc"])
    r = _CACHE["runner"]

    g = _host_prep(inputs)
    zeros = [zf() for zf in r["zeros_fns"]]  # on-device, no wire traffic
    args = [g[name] for name in r["param_names"]]
    if r["dbg_name"] is not None:
        dbgz = np.zeros((NCORES, 2), np.uint32)
        args[r["param_names"].index(r["dbg_name"])] = dbgz
    outs = r["sharded"](*args, *zeros)
    iy = r["out_names"].index("y")
    ya = outs[iy]
    shards = sorted(ya.addressable_shards, key=lambda s: s.index[0].start)
    for s in shards:
        s.data.copy_to_host_async()
    y32 = np.empty((N, COUT, H, W), np.float32)
    v = y32.reshape(NCORES, NPC, 2, 128, HW)
    for j, s in enumerate(shards):
        np.copyto(v[j], np.asarray(s.data))
    return y32
